# revision 29
# baseline (speedup 1.0000x reference)
"""Trainium2 Bass kernel for the MCA (multi-axis pooled gating) module.

Computation (per sample b):
    hw_m = mean_{u,v} x   uv_m = mean_{h,w} x   uh_m = mean_{v,w} x   vw_m = mean_{u,h} x
    body = conv2(silu(conv1(uvhw)))   (1x1 convs on the packed (H+V, W+U) pooled map)
    gates: hw_g = f0(body_hw), uv_g = f1(body_uv), uh_g = f2(body_uh), vw_g = f3(body_vw)
    out = x * (hw_g + uv_g + uh_g + vw_g)      (each gate broadcast to the 6D shape)

Distribution: 8 cores = 4 samples x 2 h-halves. Each core owns
x[b, :, :, :, hh*32:(hh+1)*32, :] (13.1 MB) resident in SBUF, so x is read from
HBM exactly once and the result is written in place over it. The only
cross-core data are the h-reduced pools (uv_m, vw_m partials), exchanged as two
small pair AllReduces (split by v so the second half overlaps the first
group's gating work).

On-core layout: SBUF partition p = hs*64 + c, where the core's 32 h-rows split
as h2 = hs*16 + hl. Pools that fully reduce h fold the hs partition halves with
a small DMA+add before the collectives.

Engine budget: DVE does the w-reduction, the final multiplies and a share of
the gate-broadcast adds; the PE does the (u,v)-pool accumulation (fp32r
identity matmuls) plus the tiny channel-mixing convs and a share of the gate
adds; GpSimd accumulates the hw pool and takes the remaining gate adds; ACT
does PSUM evacuation, scaling, SiLU and bias adds.
"""

import sys
if '/opt/trn_rl_repo' not in sys.path:
    sys.path.insert(0, '/opt/trn_rl_repo')

from contextlib import ExitStack

import numpy as np
import concourse.bass as bass
import concourse.bacc as bacc
import concourse.tile as tile
from concourse import mybir

F32 = mybir.dt.float32
F32R = mybir.dt.float32r
AF = mybir.ActivationFunctionType
ALU = mybir.AluOpType


def _ap(t_ap, dims, extra_off=0):
    """Manual free-dim view of an AP: dims = [(step_elems, count), ...]."""
    return bass.AP(
        tensor=t_ap.tensor,
        offset=t_ap.offset + extra_off,
        ap=[list(t_ap.ap[0])] + [[s, c] for (s, c) in dims],
    )


def build_program(C=64, U=5, V=5, H2=32, W=64, n_cores=8):
    """One SPMD program; per-core inputs select the (b, h-half) shard."""
    assert C == 64 and H2 % 2 == 0
    HL = H2 // 2              # h rows per hs partition group
    P = 2 * C                 # 128 partitions = (hs, c)
    CHW = HL * W              # free size of one (u,v) chunk per partition
    NMM = min(512, CHW)       # matmul moving-operand max
    NUV = U * V
    NB = U + W                # per-v partials block: [uv_u | vw_w]
    VA = max(1, (3 * V) // 5)  # v-count in the first collective group
    H = 2 * H2

    nc = bacc.Bacc('TRN2', target_bir_lowering=False, debug=False,
                   enable_asserts=False, num_devices=n_cores)

    x_d = nc.dram_tensor("x", [V, 2, C, U, HL, W], F32, kind="ExternalInput").ap()
    out_d = nc.dram_tensor("out", [V, 2, C, U, HL, W], F32, kind="ExternalOutput").ap()
    NCON = P + 6 * C + 6
    cpack_d = nc.dram_tensor("cpack", [P, NCON], F32, kind="ExternalInput").ap()
    BF16 = mybir.dt.bfloat16
    id16_d = nc.dram_tensor("id16", [P, P], BF16, kind="ExternalInput").ap()

    x_v_view = x_d
    out_v_view = out_d

    def mm(out_ps, lhsT, rhs, start, stop):
        nc.tensor.matmul(out_ps, lhsT.bitcast(F32R), rhs.bitcast(F32R),
                         start=start, stop=stop)

    mm16 = None  # bound below once id16 exists

    with tile.TileContext(nc) as tc, ExitStack() as ctx:
        consts = ctx.enter_context(tc.tile_pool(name="consts", bufs=1))
        xpool = ctx.enter_context(tc.tile_pool(name="x", bufs=V))
        sumu_pool = ctx.enter_context(tc.tile_pool(name="sumu", bufs=2))
        small = ctx.enter_context(tc.tile_pool(name="small", bufs=1))
        convp = ctx.enter_context(tc.tile_pool(name="convp", bufs=2))
        ppool = ctx.enter_context(tc.tile_pool(name="pp", bufs=U))
        gpool = ctx.enter_context(tc.tile_pool(name="gpool", bufs=3))
        phase1_ctx = ExitStack()
        ps_acc = phase1_ctx.enter_context(
            tc.tile_pool(name="ps_acc", bufs=2, space="PSUM"))
        ps_hw = phase1_ctx.enter_context(
            tc.tile_pool(name="ps_hw", bufs=1, space="PSUM"))
        dram = ctx.enter_context(tc.tile_pool(name="dram", bufs=1, space="DRAM"))

        cpack = consts.tile([P, NCON], F32)
        nc.gpsimd.dma_start(out=cpack[:].bitcast(F32R),
                            in_=cpack_d[:, :].bitcast(F32R))
        id16 = consts.tile([P, P], BF16)
        nc.gpsimd.dma_start(out=id16[:], in_=id16_d[:, :])
        ident = cpack[:, 0:P]
        # weights replicated on both hs partition halves so conv matmuls can
        # pick an lhsT whose base partition matches the rhs half
        wnames = ("w1T", "w2T", "f0T", "f1T", "f2T", "f3T")
        wt = {nm: cpack[:, P + i * C:P + (i + 1) * C]
              for i, nm in enumerate(wnames)}
        bnames = ("b1", "b2", "fb0", "fb1", "fb2", "fb3")
        bt = {nm: cpack[0:C, P + 6 * C + i:P + 6 * C + i + 1]
              for i, nm in enumerate(bnames)}

        def mm16(out_ps, rhs, start, stop):
            nc.tensor.matmul(out_ps, id16[:], rhs, start=start, stop=stop)

        # ---------------- Phase 1: load x + pools -------------------------
        partials = small.tile([P, V * NB], F32)   # per-v blocks [uv_u | vw_w]
        s_w = small.tile([P, V, U, HL], F32)      # x summed over w
        hw_ps = ps_hw.tile([P, CHW], F32)         # x summed over (u, v)
        xv_t = []

        cc_out_sb = {}

        cc_out_d = {}

        def emit_group_cc(g, v0, v1):
            """Fold hs halves of partials[v0:v1]; trigger the pair AllReduce.

            Input staging and the trigger live on GpSimd (which never has to
            wait for the collective to finish); the completion-gated output
            read is emitted separately via emit_cc_read at a point where its
            host engine is idle anyway.
            """
            sl = slice(v0 * NB, v1 * NB)
            n = (v1 - v0) * NB
            ft = small.tile([C, n], F32, name=f"fold_{g}", tag=f"fold_{g}")
            nc.gpsimd.dma_start(out=ft[:], in_=partials[C:2 * C, sl])
            ci = small.tile([C, n], F32, name=f"ccin_{g}", tag=f"ccin_{g}")
            nc.gpsimd.tensor_add(ci[:], partials[0:C, sl], ft[:])
            cid = dram.tile([C, n], F32, name=f"ccind_{g}", tag=f"ccind_{g}")
            cod = dram.tile([C, n], F32, name=f"ccoutd_{g}", tag=f"ccoutd_{g}")
            nc.gpsimd.dma_start(out=cid[:], in_=ci[:])
            groups = [[2 * i, 2 * i + 1] for i in range(n_cores // 2)]
            nc.gpsimd.collective_compute(
                "AllReduce", ALU.add, replica_groups=groups,
                ins=[cid[:].opt()], outs=[cod[:].opt()])
            cc_out_d[g] = (cod, n)

        def emit_cc_read(g):
            cod, n = cc_out_d[g]
            co = small.tile([C, n], F32, name=f"ccout_{g}", tag=f"ccout_{g}")
            nc.scalar.dma_start(out=co[:], in_=cod[:])
            cc_out_sb[g] = co

        for v in range(V):
            xv = xpool.tile([P, U, HL, W], F32, tag="xv")
            xv_t.append(xv)
            for hs in range(2):
                nc.sync.dma_start(out=xv[hs * C:(hs + 1) * C].bitcast(F32R),
                                  in_=x_v_view[v, hs].bitcast(F32R))

            acc = ps_acc.tile([P, CHW], F32, tag="acc")   # sum over u, this v
            for u in range(U):
                x16 = sumu_pool.tile([P, CHW], mybir.dt.bfloat16, tag="x16")
                nc.scalar.copy(out=x16[:],
                               in_=xv[:, u].rearrange("p hl w -> p (hl w)"))
                for j0 in range(0, CHW, NMM):
                    mm16(acc[:, j0:j0 + NMM], x16[:, j0:j0 + NMM],
                       start=(u == 0), stop=(u == U - 1))
            # vw partial: reduce hl out of acc -> [P, W]
            accv = acc[:].rearrange("p (hl w) -> p w hl", hl=HL)
            nc.vector.tensor_reduce(partials[:, v * NB + U:(v + 1) * NB],
                                    accv, axis=mybir.AxisListType.X, op=ALU.add)
            # s_w then uv partial for this v
            nc.vector.tensor_reduce(s_w[:, v], xv[:],
                                    axis=mybir.AxisListType.X, op=ALU.add)
            nc.vector.tensor_reduce(partials[:, v * NB:v * NB + U], s_w[:, v],
                                    axis=mybir.AxisListType.X, op=ALU.add)
            # hw accumulation: acc (copied to SBUF) back through the PE
            sumu = sumu_pool.tile([P, CHW], mybir.dt.bfloat16, tag="sumu")
            nc.scalar.copy(out=sumu[:], in_=acc[:])
            for j0 in range(0, CHW, NMM):
                mm16(hw_ps[:, j0:j0 + NMM], sumu[:, j0:j0 + NMM],
                   start=(v == 0), stop=(v == V - 1))
            if v == VA - 1:
                emit_group_cc("A", 0, VA)
            elif v == V - 1:
                emit_group_cc("B", VA, V)

        # uh local sums -> means
        uh_raw = small.tile([P, U, HL], F32)
        swv = s_w[:].rearrange("p v u hl -> p u hl v")
        nc.vector.tensor_reduce(uh_raw[:], swv, axis=mybir.AxisListType.X,
                                op=ALU.add)
        uh_sc = small.tile([P, U, HL], F32)
        nc.scalar.activation(out=uh_sc[:].bitcast(F32R), in_=uh_raw[:],
                             func=AF.Copy, scale=1.0 / (V * W))
        # hw means
        hw_m = small.tile([P, CHW], F32)
        nc.scalar.activation(out=hw_m[:].bitcast(F32R), in_=hw_ps[:],
                             func=AF.Copy, scale=1.0 / NUV)
        phase1_ctx.close()   # release pool-phase PSUM banks
        ps1p = ctx.enter_context(tc.tile_pool(name="ps1p", bufs=3, space="PSUM"))
        ps2p = ctx.enter_context(tc.tile_pool(name="ps2p", bufs=2, space="PSUM"))
        ps3p = ctx.enter_context(tc.tile_pool(name="ps3p", bufs=2, space="PSUM"))

        # gate buffers (same pixel orders as the conv inputs)
        hwg = small.tile([P, CHW], F32)          # (hl, w) per (hs,c) partition
        uhg = small.tile([P, U * HL], F32)       # (u, hl) per (hs,c) partition
        vwg = small.tile([P, V * W], F32)        # (v, w), replicated over hs
        uvg = small.tile([P, NUV], F32)          # (v, u), replicated over hs
        uv_sc = small.tile([C, NUV + 1], F32)    # (v,u) order (+1 pad col)
        vw_sc = small.tile([C, V * W], F32)      # (v,w) order

        def run_conv_jobs(jobs):
            """Software-pipelined 1x1-conv chains (2 jobs in flight).

            Each job: (rhs_ap, nn, hs, f_nm, fb_nm, store). Chain per job:
            u1 = w1 @ rhs ; a1 = silu(u1 + b1) ; u2 = w2 @ a1 + b2 ;
            gate = f @ u2 + fb.
            """
            ps1s = [None] * len(jobs)
            for j in range(len(jobs) + 2):
                if j < len(jobs):
                    rhs, nn, hs, f_nm, fb_nm, store = jobs[j]
                    w_sl = slice(hs * C, (hs + 1) * C)
                    ps1 = ps1p.tile([C, nn], F32, tag="ps1")
                    mm(ps1[:], wt["w1T"][w_sl, :], rhs, start=True, stop=True)
                    ps1s[j] = ps1
                k = j - 2
                if k < 0 or k >= len(jobs):
                    continue
                rhs, nn, hs, f_nm, fb_nm, store = jobs[k]
                ps1 = ps1s[k]
                sig = convp.tile([C, nn], F32, tag="sig")
                nc.scalar.activation(out=sig[:], in_=ps1[:], func=AF.Sigmoid,
                                     bias=bt["b1"])
                a1 = convp.tile([C, nn], F32, tag="a1")
                nc.vector.scalar_tensor_tensor(
                    out=a1[:].bitcast(F32R), in0=ps1[:], scalar=bt["b1"],
                    in1=sig[:], op0=ALU.add, op1=ALU.mult)
                ps2 = ps2p.tile([C, nn], F32, tag="ps2")
                mm(ps2[:], wt["w2T"][0:C, :], a1[:], start=True, stop=True)
                body = convp.tile([C, nn], F32, tag="body")
                nc.vector.tensor_scalar(out=body[:].bitcast(F32R), in0=ps2[:],
                                        scalar1=bt["b2"], scalar2=None,
                                        op0=ALU.add)
                ps3 = ps3p.tile([C, nn], F32, tag="ps3")
                mm(ps3[:], wt[f_nm][0:C, :], body[:], start=True, stop=True)
                gate = convp.tile([C, nn], F32, tag="gate")
                nc.scalar.activation(out=gate[:], in_=ps3[:], func=AF.Identity,
                                     bias=bt[fb_nm])
                store(gate)

        # local jobs: hw (per hs, per 512-chunk) and uh (per hs)
        jobs = []
        for hs in range(2):
            for j0 in range(0, CHW, NMM):
                def st_hw(gate, hs=hs, j0=j0):
                    nc.sync.dma_start(out=hwg[hs * C:(hs + 1) * C,
                                              j0:j0 + NMM], in_=gate[:])
                jobs.append((hw_m[hs * C:(hs + 1) * C, j0:j0 + NMM], NMM, hs,
                             "f0T", "fb0", st_hw))
        for hs in range(2):
            def st_uh(gate, hs=hs):
                nc.sync.dma_start(out=uhg[hs * C:(hs + 1) * C, :], in_=gate[:])
            jobs.append((uh_sc[hs * C:(hs + 1) * C], U * HL, hs,
                         "f2T", "fb2", st_uh))
        run_conv_jobs(jobs)

        # P_u = hwg + uhg[:, u, :] broadcast over w (local; before any cc dep)
        pbufs = []
        for u in range(U):
            pbuf = ppool.tile([P, CHW], F32, tag="p")
            uh_b = _ap(uhg[:], [(1, HL), (0, W)], extra_off=u * HL)
            nc.vector.tensor_add(pbuf[:], hwg[:], uh_b)
            pbufs.append(pbuf)

        def scale_group(v0, v1, g):
            co = cc_out_sb[g]
            cnt = v1 - v0
            uv_src = _ap(co[:], [(NB, cnt), (1, U)])
            nc.vector.tensor_scalar(
                out=uv_sc[:, v0 * U:v1 * U].bitcast(F32R), in0=uv_src,
                scalar1=1.0 / (H * W), scalar2=None, op0=ALU.mult)
            vw_src = _ap(co[:], [(NB, cnt), (1, W)], extra_off=U)
            nc.vector.tensor_scalar(
                out=vw_sc[:, v0 * W:v1 * W].bitcast(F32R), in0=vw_src,
                scalar1=1.0 / (U * H), scalar2=None, op0=ALU.mult)
            if v1 == V and NUV % 2:   # pad col so uv widths stay even
                nc.vector.tensor_scalar(
                    out=uv_sc[:, NUV:NUV + 1].bitcast(F32R), in0=co[:, 0:1],
                    scalar1=1.0, scalar2=None, op0=ALU.mult)

        def group_jobs(v0, v1):
            jb = []
            nvw = (v1 - v0) * W

            def st_vw(gate, v0=v0, nvw=nvw):
                nc.sync.dma_start(out=vwg[0:C, v0 * W:v0 * W + nvw],
                                  in_=gate[:])
                nc.sync.dma_start(out=vwg[C:2 * C, v0 * W:v0 * W + nvw],
                                  in_=gate[:])
            jb.append((vw_sc[:, v0 * W:v1 * W], nvw, 0, "f3T", "fb3", st_vw))
            nuv = (v1 - v0) * U
            pad = nuv % 2

            def st_uv(gate, v0=v0, nuv=nuv):
                nc.sync.dma_start(out=uvg[0:C, v0 * U:v0 * U + nuv],
                                  in_=gate[:, 0:nuv])
                nc.sync.dma_start(out=uvg[C:2 * C, v0 * U:v0 * U + nuv],
                                  in_=gate[:, 0:nuv])
            jb.append((uv_sc[:, v0 * U:v1 * U + pad], nuv + pad, 0,
                       "f1T", "fb1", st_uv))
            return jb

        def emit_q(v0, v1):
            cnt = v1 - v0
            vw_b = _ap(vwg[:], [(W, cnt), (0, U), (1, W)], extra_off=v0 * W)
            uv_b = _ap(uvg[:], [(U, cnt), (1, U), (0, W)], extra_off=v0 * U)
            nc.vector.tensor_add(qbuf[:, v0:v1], vw_b, uv_b)

        def emit_phase3(v0, v1):
            for v in range(v0, v1):
                for u in range(U):
                    xin = xv_t[v][:, u].rearrange("p hl w -> p (hl w)")
                    q_b = _ap(qbuf[:], [(0, HL), (1, W)],
                              extra_off=(v * U + u) * W)
                    if (v * U + u) % 2 == 0:
                        g = gpool.tile([P, CHW], F32, tag="g_gp", bufs=3)
                        nc.gpsimd.tensor_add(g[:], pbufs[u][:], q_b)
                    else:
                        g = gpool.tile([P, CHW], F32, tag="g_dve", bufs=2)
                        nc.vector.tensor_add(g[:], pbufs[u][:], q_b)
                    nc.vector.tensor_mul(xin.bitcast(F32R), xin, g[:])
                for hs in range(2):
                    nc.sync.dma_start(out=out_v_view[v, hs],
                                      in_=xv_t[v][hs * C:(hs + 1) * C])

        qbuf = small.tile([P, V, U, W], F32)

        # group A: scales + convs + Q, then its gating while B is in flight
        emit_cc_read("A")
        scale_group(0, VA, "A")
        run_conv_jobs(group_jobs(0, VA))
        emit_q(0, VA)
        emit_cc_read("B")
        emit_phase3(0, VA)
        # group B: scales + convs + Q + gating
        scale_group(VA, V, "B")
        run_conv_jobs(group_jobs(VA, V))
        emit_q(VA, V)
        emit_phase3(VA, V)

    nc.compile()
    return nc


# ---------------------------------------------------------------------------
# Host entry point (full problem size, 8 cores)

B, C, U, V, H, W = 4, 64, 5, 5, 64, 64
H2 = H // 2

_prog_cache = {}


def _get_prog():
    if "nc" not in _prog_cache:
        _prog_cache["nc"] = build_program(C=C, U=U, V=V, H2=H2, W=W, n_cores=8)
    return _prog_cache["nc"]


def make_const_pack(inputs):
    import ml_dtypes
    P = 2 * C
    ws = [np.asarray(inputs["w1"], np.float32).T,
          np.asarray(inputs["w2"], np.float32).T]
    bs = [np.asarray(inputs["b1"], np.float32),
          np.asarray(inputs["b2"], np.float32)]
    for i in range(4):
        ws.append(np.asarray(inputs[f"fw{i}"], np.float32).T)
        bs.append(np.asarray(inputs[f"fb{i}"], np.float32))
    ws = [ws[0], ws[1], ws[2], ws[3], ws[4], ws[5]]
    # column layout: [ident(P) | 6 weights (C each, hs-replicated) | 6 biases]
    ncon = P + 6 * C + 6
    cpack = np.zeros((P, ncon), dtype=np.float32)
    cpack[:, 0:P] = np.eye(P, dtype=np.float32)
    # reorder: w1T, w2T, f0T..f3T as in the device program
    order = [ws[0], ws[1], ws[2], ws[3], ws[4], ws[5]]
    for i, w in enumerate(order):
        blk = np.vstack([w, w])        # replicate across hs halves
        cpack[:, P + i * C:P + (i + 1) * C] = blk
    border = [bs[0], bs[1], bs[2], bs[3], bs[4], bs[5]]
    for i, b in enumerate(border):
        cpack[:, P + 6 * C + i] = np.concatenate([b, b])
    id16 = np.eye(P, dtype=ml_dtypes.bfloat16)
    return cpack, id16


def make_in_maps(inputs):
    x = np.asarray(inputs["x"], dtype=np.float32)
    cpack, id16 = make_const_pack(inputs)
    base = {"cpack": cpack, "id16": id16}

    HL = H2 // 2
    in_maps = []
    for core in range(8):
        b, hh = core // 2, core % 2
        s6 = x[b, :, :, :, hh * H2:(hh + 1) * H2, :]
        arr = np.ascontiguousarray(
            s6.reshape(C, U, V, 2, HL, W).transpose(2, 3, 0, 1, 4, 5))
        in_maps.append({"x": arr, **base})
    return in_maps


def assemble_out(results):
    HL = H2 // 2
    out = np.empty((B, C, U, V, H, W), dtype=np.float32)
    for core in range(8):
        b, hh = core // 2, core % 2
        r = results[core]["out"]          # [V, 2, C, U, HL, W]
        out[b, :, :, :, hh * H2:(hh + 1) * H2, :] = (
            r.transpose(2, 3, 0, 1, 4, 5).reshape(C, U, V, H2, W))
    return out


def kernel(**inputs):
    from concourse.bass_utils import run_bass_kernel_spmd

    in_maps = make_in_maps(inputs)
    nc = _get_prog()
    res = run_bass_kernel_spmd(nc, in_maps, core_ids=list(range(8)))
    return assemble_out(res.results)
